# revision 17
# baseline (speedup 1.0000x reference)
"""CharBiLSTM Trainium2 kernel.

Full inputs in, full output out. Shards the 4096 words across 8 NeuronCores
(data parallel, weights replicated), runs a Bass/Tile kernel per core, and
reassembles the [B, W, H] output on the host.

Strategy:
  - Sort words by length (desc) and deal them round-robin so every core gets
    an identical per-length-class profile (padding with a few dummy words),
    letting one SPMD program serve all 8 cores.
  - Keep LSTM state transposed (h^T: [H(part), words(free)]) so the
    recurrence matmul Whh @ h^T needs no transposes anywhere.
  - At global step t only words with len > t are active; sorted desc these
    form a prefix, so each step is a dense matmul over a shrinking prefix.
    Forward walks t = 0..15 (prefix shrinks), backward walks t = 15..0
    (prefix grows from zero-initialized state). Inactive columns are simply
    not touched: after the loop h_fwd / h_bwd sit in the state tiles.
  - Char embedding lookup is a host-side gather producing xT [E=128, cols]
    packed per step; fwd and bwd share the same columns.
  - Matmul operands in fp16 (full PE rate, ~tf32-grade mantissa for this
    data's range, FWL weight loads); PSUM accumulation and the cell state c
    stay fp32. Gate bias + sigmoid/tanh fused into the PSUM->SBUF
    evacuation on ScalarE; cell update on VectorE.
"""

import sys

if '/opt/trn_rl_repo' not in sys.path:
    sys.path.insert(0, '/opt/trn_rl_repo')

import warnings

warnings.filterwarnings('ignore')

import numpy as np

# Problem dims (hardcoded per spec)
B, W, L = 64, 64, 16
E, H, V = 128, 512, 256
N = B * W
N_CORES = 8
KC = H // 128          # Whh contraction chunks
MC = (4 * H) // 128    # gate-row chunks (16); gate g occupies chunks 4g..4g+3
P = 128

MM_DT = 'float16'      # matmul dtype: 'float16' | 'bfloat16' | 'float32r'


# ---------------------------------------------------------------------------
# Host-side scheduling
# ---------------------------------------------------------------------------

def _plan(word_lens_flat):
    """Deal words to cores with equalized length profiles.

    Returns (per_core_words, counts):
      per_core_words: [8][M_pad] global word id or -1 for dummy, sorted by
        descending length class.
      counts: c_t = number of words (per core) with len > t, t = 0..L-1.
    """
    lens = np.asarray(word_lens_flat)
    ids_by_class = {v: [] for v in range(1, L + 1)}
    order = np.argsort(-lens, kind='stable')
    for wid in order:
        ids_by_class[int(lens[wid])].append(int(wid))

    m_v = {v: (len(ids_by_class[v]) + N_CORES - 1) // N_CORES
           for v in range(1, L + 1)}

    per_core = [[] for _ in range(N_CORES)]
    rot = 0
    for v in range(L, 0, -1):
        ids = ids_by_class[v]
        buckets = [[] for _ in range(N_CORES)]
        for i, wid in enumerate(ids):
            buckets[(rot + i) % N_CORES].append(wid)
        rot = (rot + len(ids)) % N_CORES
        for k in range(N_CORES):
            b = buckets[k]
            b += [-1] * (m_v[v] - len(b))
            per_core[k].extend(b)

    counts = [sum(m_v[v] for v in range(t + 1, L + 1)) for t in range(L)]
    return per_core, counts


def _prep_weights(Wih, Whh, bih, bhh, mmnp):
    """Repack weights into the SBUF layouts the kernel uses."""
    whh_sb = np.ascontiguousarray(
        Whh.T.reshape(KC, P, 4 * H).transpose(1, 0, 2)).astype(mmnp)
    wih_sb = np.ascontiguousarray(Wih.T).astype(mmnp)               # [128, 2048]
    bias_sb = np.ascontiguousarray(
        (bih + bhh).reshape(MC, P).T).astype(np.float32)            # [128, 16]
    return whh_sb, wih_sb, bias_sb


# ---------------------------------------------------------------------------
# Device program
# ---------------------------------------------------------------------------

def _build_program(counts, m_pad, xcols):
    import concourse.bacc as bacc
    import concourse.mybir as mybir
    import concourse.tile as tile

    f32 = mybir.dt.float32
    mdt = getattr(mybir.dt, MM_DT)
    is_r = MM_DT == 'float32r'
    SIG = mybir.ActivationFunctionType.Sigmoid
    TANH = mybir.ActivationFunctionType.Tanh

    off = np.concatenate([[0], np.cumsum(counts)]).astype(int)  # per-step col offset

    nc = bacc.Bacc(None, target_bir_lowering=False)

    xT_d = nc.dram_tensor("xT", [P, xcols], mdt, kind="ExternalInput")
    whh_d = {d: nc.dram_tensor(f"whh_{d}", [P, KC, 4 * H], mdt, kind="ExternalInput")
             for d in ("f", "b")}
    wih_d = {d: nc.dram_tensor(f"wih_{d}", [P, 4 * H], mdt, kind="ExternalInput")
             for d in ("f", "b")}
    bias_d = {d: nc.dram_tensor(f"bias_{d}", [P, MC], f32, kind="ExternalInput")
              for d in ("f", "b")}
    wp_d = nc.dram_tensor("wp", [P, 8, H], mdt, kind="ExternalInput")
    bp_d = nc.dram_tensor("bp", [P, H], f32, kind="ExternalInput")
    out_d = nc.dram_tensor("out", [m_pad, H], f32, kind="ExternalOutput")

    TW = m_pad  # state tile width

    def col_blocks(c):
        """Split step width c into matmul blocks (base, width, n_padded).

        Widths > 512 (PSUM bank limit) split into two near-equal halves so
        no block is tiny (tiny N leaves the weight load exposed)."""
        if c <= 512:
            widths = [c]
        else:
            widths = [(c + 1) // 2, c // 2]
        blocks = []
        base = 0
        for bw in widths:
            npad = max(bw, 256) if (is_r and base == 0) else bw
            blocks.append((base, bw, npad))
            base += bw
        return blocks

    with tile.TileContext(nc) as tc:
        with tc.tile_pool(name="persist", bufs=1) as pp:
            # DMAs in first-use order: the data for the first matmul group
            # (fwd gate i, first column chunk) lands first.
            whh = {d: pp.tile([P, KC, 4 * H], mdt, name=f"whh{d}") for d in ("f", "b")}
            wih = {d: pp.tile([P, 4 * H], mdt, name=f"wih{d}") for d in ("f", "b")}
            bias = {d: pp.tile([P, MC], f32, name=f"bias{d}") for d in ("f", "b")}
            xT = pp.tile([P, xcols], mdt)

            nc.sync.dma_start(out=bias["f"], in_=bias_d["f"][:, :])
            nc.sync.dma_start(out=bias["b"], in_=bias_d["b"][:, :])
            x0 = min(1024, xcols)
            nc.sync.dma_start(out=wih["f"], in_=wih_d["f"][:, :])
            nc.scalar.dma_start(out=xT[:, :x0], in_=xT_d[:, :x0])
            nc.scalar.dma_start(out=wih["b"], in_=wih_d["b"][:, :])
            # first non-trivial steps touch every gate of both directions in
            # emission order i, g, f, o alternating fwd/bwd; spread across
            # issuing engines for more HW-DGE queue parallelism.
            for g_i, gname in ((0, 'i'), (2, 'g'), (1, 'f'), (3, 'o')):
                gs = slice(g_i * H, (g_i + 1) * H)
                nc.sync.dma_start(out=whh["f"][:, :, gs], in_=whh_d["f"][:, :, gs])
                nc.scalar.dma_start(out=whh["b"][:, :, gs], in_=whh_d["b"][:, :, gs])
            nc.scalar.dma_start(out=xT[:, x0:], in_=xT_d[:, x0:])
            wp = pp.tile([P, 8, H], mdt)
            nc.sync.dma_start(out=wp, in_=wp_d[:, :, :])
            bp = pp.tile([P, H], f32)
            nc.sync.dma_start(out=bp, in_=bp_d[:, :])

            hS = {}
            cS = {}
            for d in ("f", "b"):
                hS[d] = pp.tile([P, KC, TW], mdt, name=f"h{d}")
                cS[d] = pp.tile([P, KC, TW], f32, name=f"c{d}")
            # Each direction's first step skips the recurrence entirely, so
            # fwd state needs no zero-init (every read is covered by a prior
            # write). bwd needs zeros for columns entering mid-sequence.
            nc.gpsimd.memset(hS["b"][:, :, :].bitcast(f32 if is_r else mdt), 0.0)
            nc.gpsimd.memset(cS["b"][:, :, :], 0.0)

            def dir_step(d, t, first=False):
                # On a direction's first step h = c = 0: the Whh matmuls
                # vanish and the f gate is unused (c = sig_i * tanh_g).
                c = counts[t]
                gates = (((0, 'i'), (2, 'g'), (3, 'o')) if first else
                         ((0, 'i'), (2, 'g'), (1, 'f'), (3, 'o')))
                for (bs, bw, npad) in col_blocks(c):
                    xs = off[t] + bs
                    sb = {}
                    # PE: gate matmuls; ScalarE evacuates with fused
                    # bias + nonlinearity per 128-row chunk. PSUM tiles are
                    # bank-granular so banks recycle as soon as each chunk
                    # is evacuated.
                    for g, gname in gates:
                        gt = gp.tile([P, 4, 512], mdt, name=f"sb{gname}",
                                     tag=f"g{gname}", bufs=2)
                        func = TANH if g == 2 else SIG
                        for j in range(4):
                            m = g * 4 + j
                            ms = slice(m * P, (m + 1) * P)
                            ps = psp.tile([P, 512], f32, name=f"ps{gname}{j}",
                                          tag="ps", bufs=8)
                            if not first:
                                for k in range(KC):
                                    nc.tensor.matmul(
                                        ps[:, :npad],
                                        whh[d][:, k, ms],
                                        hS[d][:, k, bs:bs + npad],
                                        start=(k == 0), stop=False)
                            nc.tensor.matmul(
                                ps[:, :npad],
                                wih[d][:, ms],
                                xT[:, xs:xs + npad],
                                start=first, stop=True)
                            nc.scalar.activation(
                                gt[:, j, :bw], ps[:, :bw], func,
                                bias=bias[d][:, m:m + 1], scale=1.0)
                        sb[gname] = gt
                    # VectorE cell update, split into two H-chunk halves so
                    # the next step's k-chunk matmuls unblock sooner:
                    #   t1 = sig_i * tanh_g   (in-place into sig_i, fp16 2x)
                    #   c  = sig_f * c + t1   (fp32; first step: c = t1)
                    #   h  = sig_o * tanh(c)  (fp16 out)
                    tc_t = gp.tile([P, 4, 512], mdt, name="tanhc", tag="tc",
                                   bufs=2)
                    for jh in (slice(0, 2), slice(2, 4)):
                        csl = cS[d][:, jh, bs:bs + bw]
                        hsl = hS[d][:, jh, bs:bs + bw]
                        if first:
                            nc.vector.tensor_mul(csl, sb['i'][:, jh, :bw],
                                                 sb['g'][:, jh, :bw])
                        else:
                            nc.vector.tensor_mul(sb['i'][:, jh, :bw],
                                                 sb['i'][:, jh, :bw],
                                                 sb['g'][:, jh, :bw])
                            nc.vector.tensor_mul(csl, sb['f'][:, jh, :bw], csl)
                            nc.vector.tensor_add(csl, csl, sb['i'][:, jh, :bw])
                        nc.scalar.activation(tc_t[:, jh, :bw], csl, TANH)
                        nc.vector.tensor_mul(hsl, sb['o'][:, jh, :bw],
                                             tc_t[:, jh, :bw])

            with tc.tile_pool(name="gates", bufs=2) as gp, \
                 tc.tile_pool(name="psum", bufs=2, space="PSUM") as psp:
                for r in range(L):
                    dir_step('f', r, first=(r == 0))
                    dir_step('b', L - 1 - r, first=(r == 0))

            # Projection: out[words, H] = [h_bwd; h_fwd]^T @ Wp^T + bp
            with tc.tile_pool(name="proj", bufs=2) as prp, \
                 tc.tile_pool(name="prps", bufs=2, space="PSUM") as prps:
                nmc = (m_pad + P - 1) // P
                for mi in range(nmc):
                    pw = min(P, m_pad - mi * P)
                    po = prps.tile([P, H], f32, name="po", tag="po")
                    for kk in range(8):
                        src = hS['b'] if kk < 4 else hS['f']
                        nc.tensor.matmul(
                            po[:pw, :],
                            src[:, kk % 4, mi * P:mi * P + pw],
                            wp[:, kk, :],
                            start=(kk == 0), stop=(kk == 7))
                    ot = prp.tile([P, H], f32, name="ot", tag="ot")
                    nc.vector.tensor_add(ot[:pw, :], po[:pw, :], bp[:pw, :])
                    nc.sync.dma_start(out=out_d[mi * P:mi * P + pw, :],
                                      in_=ot[:pw, :])

    nc.compile()
    return nc


# ---------------------------------------------------------------------------
# Entry points
# ---------------------------------------------------------------------------

def _run(inputs, trace=False, tmpdir=None, reps=1):
    from concourse.bass_utils import run_bass_kernel_spmd

    mmnp = {'float16': np.float16, 'bfloat16': None,
            'float32r': np.float32}.get(MM_DT)
    if mmnp is None:
        import ml_dtypes
        mmnp = ml_dtypes.bfloat16

    chars = np.asarray(inputs["chars"]).reshape(N, L)
    lens = np.asarray(inputs["word_lens"]).reshape(N).astype(np.int64)
    emb = np.asarray(inputs["emb"], dtype=np.float32)

    per_core, counts = _plan(lens)
    m_pad = len(per_core[0])
    ctot = int(sum(counts))
    xcols = ctot + 512

    whh_f, wih_f, bias_f = _prep_weights(
        np.asarray(inputs["Wih_f"], np.float32), np.asarray(inputs["Whh_f"], np.float32),
        np.asarray(inputs["bih_f"], np.float32), np.asarray(inputs["bhh_f"], np.float32),
        mmnp)
    whh_b, wih_b, bias_b = _prep_weights(
        np.asarray(inputs["Wih_b"], np.float32), np.asarray(inputs["Whh_b"], np.float32),
        np.asarray(inputs["bih_b"], np.float32), np.asarray(inputs["bhh_b"], np.float32),
        mmnp)
    Wp = np.asarray(inputs["Wp"], np.float32)
    bp = np.asarray(inputs["bp"], np.float32)
    wp_sb = np.ascontiguousarray(Wp.T.reshape(8, P, H).transpose(1, 0, 2)).astype(mmnp)
    bp_sb = np.ascontiguousarray(np.tile(bp[None, :], (P, 1))).astype(np.float32)

    embT = np.ascontiguousarray(emb.T)  # [128, 256]

    in_maps = []
    for k in range(N_CORES):
        wl = per_core[k]
        cw = np.zeros((m_pad, L), dtype=np.int64)
        for r, wid in enumerate(wl):
            if wid >= 0:
                cw[r] = chars[wid]
        idx = np.concatenate([cw[:counts[t], t] for t in range(L)])
        xT = np.zeros((P, xcols), dtype=mmnp)
        xT[:, :ctot] = embT[:, idx].astype(mmnp)
        in_maps.append(dict(
            xT=xT, whh_f=whh_f, whh_b=whh_b, wih_f=wih_f, wih_b=wih_b,
            bias_f=bias_f, bias_b=bias_b, wp=wp_sb, bp=bp_sb))

    nc = _build_program(counts, m_pad, xcols)
    times = []
    for r in range(reps):
        last = r == reps - 1
        td = (tmpdir + f"_{r}") if (tmpdir and trace) else tmpdir
        res = run_bass_kernel_spmd(nc, in_maps, list(range(N_CORES)),
                                   trace=trace, tmpdir=td)
        times.append(res.exec_time_ns)
    res.all_exec_times = times

    out = np.zeros((N, H), dtype=np.float32)
    for k in range(N_CORES):
        ok = res.results[k]["out"]
        for r, wid in enumerate(per_core[k]):
            if wid >= 0:
                out[wid] = ok[r]
    return out.reshape(B, W, H), res


def kernel(**inputs):
    out, _ = _run(inputs, trace=False)
    return out


# revision 18
# speedup vs baseline: 1.1926x; 1.1926x over previous
"""CharBiLSTM Trainium2 kernel.

Full inputs in, full output out. Shards the 4096 words across 8 NeuronCores
(data parallel, weights replicated), runs a Bass/Tile kernel per core, and
reassembles the [B, W, H] output on the host.

Strategy:
  - Sort words by length (desc) and deal them round-robin so every core gets
    an identical per-length-class profile (padding with a few dummy words),
    letting one SPMD program serve all 8 cores.
  - Keep LSTM state transposed (h^T: [H(part), words(free)]) so the
    recurrence matmul Whh @ h^T needs no transposes anywhere.
  - At global step t only words with len > t are active; sorted desc these
    form a prefix, so each step is a dense matmul over a shrinking prefix.
    Forward walks t = 0..15 (prefix shrinks), backward walks t = 15..0
    (prefix grows from zero-initialized state). Inactive columns are simply
    not touched: after the loop h_fwd / h_bwd sit in the state tiles.
  - Char embedding lookup is a host-side gather producing xT [E=128, cols]
    packed per step; fwd and bwd share the same columns.
  - Matmul operands in fp16 (full PE rate, ~tf32-grade mantissa for this
    data's range, FWL weight loads); PSUM accumulation and the cell state c
    stay fp32. Gate bias + sigmoid/tanh fused into the PSUM->SBUF
    evacuation on ScalarE; cell update on VectorE.
"""

import sys

if '/opt/trn_rl_repo' not in sys.path:
    sys.path.insert(0, '/opt/trn_rl_repo')

import warnings

warnings.filterwarnings('ignore')

import numpy as np

# Problem dims (hardcoded per spec)
B, W, L = 64, 64, 16
E, H, V = 128, 512, 256
N = B * W
N_CORES = 8
KC = H // 128          # Whh contraction chunks
MC = (4 * H) // 128    # gate-row chunks (16); gate g occupies chunks 4g..4g+3
P = 128

MM_DT = 'float16'      # matmul dtype: 'float16' | 'bfloat16' | 'float32r'


# ---------------------------------------------------------------------------
# Host-side scheduling
# ---------------------------------------------------------------------------

def _plan(word_lens_flat):
    """Deal words to cores with equalized length profiles.

    Returns (per_core_words, counts):
      per_core_words: [8][M_pad] global word id or -1 for dummy, sorted by
        descending length class.
      counts: c_t = number of words (per core) with len > t, t = 0..L-1.
    """
    lens = np.asarray(word_lens_flat)
    ids_by_class = {v: [] for v in range(1, L + 1)}
    order = np.argsort(-lens, kind='stable')
    for wid in order:
        ids_by_class[int(lens[wid])].append(int(wid))

    m_v = {v: (len(ids_by_class[v]) + N_CORES - 1) // N_CORES
           for v in range(1, L + 1)}

    per_core = [[] for _ in range(N_CORES)]
    rot = 0
    for v in range(L, 0, -1):
        ids = ids_by_class[v]
        buckets = [[] for _ in range(N_CORES)]
        for i, wid in enumerate(ids):
            buckets[(rot + i) % N_CORES].append(wid)
        rot = (rot + len(ids)) % N_CORES
        for k in range(N_CORES):
            b = buckets[k]
            b += [-1] * (m_v[v] - len(b))
            per_core[k].extend(b)

    counts = [sum(m_v[v] for v in range(t + 1, L + 1)) for t in range(L)]
    return per_core, counts


def _prep_weights(Wih, Whh, bih, bhh, mmnp):
    """Repack weights into the SBUF layouts the kernel uses."""
    whh_sb = np.ascontiguousarray(
        Whh.T.reshape(KC, P, 4 * H).transpose(1, 0, 2)).astype(mmnp)
    wih_sb = np.ascontiguousarray(Wih.T).astype(mmnp)               # [128, 2048]
    bias_sb = np.ascontiguousarray(
        (bih + bhh).reshape(MC, P).T).astype(np.float32)            # [128, 16]
    return whh_sb, wih_sb, bias_sb


# ---------------------------------------------------------------------------
# Device program
# ---------------------------------------------------------------------------

def _build_program(counts, m_pad, xcols):
    import concourse.bacc as bacc
    import concourse.mybir as mybir
    import concourse.tile as tile

    f32 = mybir.dt.float32
    mdt = getattr(mybir.dt, MM_DT)
    is_r = MM_DT == 'float32r'
    SIG = mybir.ActivationFunctionType.Sigmoid
    TANH = mybir.ActivationFunctionType.Tanh

    off = np.concatenate([[0], np.cumsum(counts)]).astype(int)  # per-step col offset

    nc = bacc.Bacc(None, target_bir_lowering=False)

    xT_d = nc.dram_tensor("xT", [P, xcols], mdt, kind="ExternalInput")
    whh_d = {d: nc.dram_tensor(f"whh_{d}", [P, KC, 4 * H], mdt, kind="ExternalInput")
             for d in ("f", "b")}
    wih_d = {d: nc.dram_tensor(f"wih_{d}", [P, 4 * H], mdt, kind="ExternalInput")
             for d in ("f", "b")}
    bias_d = {d: nc.dram_tensor(f"bias_{d}", [P, MC], f32, kind="ExternalInput")
              for d in ("f", "b")}
    wp_d = nc.dram_tensor("wp", [P, 8, H], mdt, kind="ExternalInput")
    bp_d = nc.dram_tensor("bp", [P, H], f32, kind="ExternalInput")
    out_d = nc.dram_tensor("out", [m_pad, H], f32, kind="ExternalOutput")

    TW = m_pad  # state tile width

    def col_blocks(c):
        """Split step width c into matmul blocks (base, width, n_padded).

        Widths > 512 (PSUM bank limit) split into two near-equal halves so
        no block is tiny (tiny N leaves the weight load exposed)."""
        if c <= 512:
            widths = [c]
        else:
            widths = [(c + 1) // 2, c // 2]
        blocks = []
        base = 0
        for bw in widths:
            npad = max(bw, 256) if (is_r and base == 0) else bw
            blocks.append((base, bw, npad))
            base += bw
        return blocks

    with tile.TileContext(nc) as tc:
        with tc.tile_pool(name="persist", bufs=1) as pp:
            # DMAs in first-use order: the data for the first matmul group
            # (fwd gate i, first column chunk) lands first.
            whh = {d: pp.tile([P, KC, 4 * H], mdt, name=f"whh{d}") for d in ("f", "b")}
            wih = {d: pp.tile([P, 4 * H], mdt, name=f"wih{d}") for d in ("f", "b")}
            bias = {d: pp.tile([P, MC], f32, name=f"bias{d}") for d in ("f", "b")}
            xT = pp.tile([P, xcols], mdt)

            nc.sync.dma_start(out=bias["f"], in_=bias_d["f"][:, :])
            nc.sync.dma_start(out=bias["b"], in_=bias_d["b"][:, :])
            x0 = min(1024, xcols)
            nc.sync.dma_start(out=wih["f"], in_=wih_d["f"][:, :])
            nc.sync.dma_start(out=xT[:, :x0], in_=xT_d[:, :x0])
            nc.sync.dma_start(out=wih["b"], in_=wih_d["b"][:, :])
            # first non-trivial steps touch every gate of both directions in
            # emission order i, g, f, o alternating fwd/bwd; spread across
            # issuing engines for more HW-DGE queue parallelism.
            for g_i, gname in ((0, 'i'), (2, 'g'), (1, 'f'), (3, 'o')):
                gs = slice(g_i * H, (g_i + 1) * H)
                nc.sync.dma_start(out=whh["f"][:, :, gs], in_=whh_d["f"][:, :, gs])
                nc.sync.dma_start(out=whh["b"][:, :, gs], in_=whh_d["b"][:, :, gs])
            nc.sync.dma_start(out=xT[:, x0:], in_=xT_d[:, x0:])
            wp = pp.tile([P, 8, H], mdt)
            nc.sync.dma_start(out=wp, in_=wp_d[:, :, :])
            bp = pp.tile([P, H], f32)
            nc.sync.dma_start(out=bp, in_=bp_d[:, :])

            hS = {}
            cS = {}
            for d in ("f", "b"):
                hS[d] = pp.tile([P, KC, TW], mdt, name=f"h{d}")
                cS[d] = pp.tile([P, KC, TW], f32, name=f"c{d}")
            # Each direction's first step skips the recurrence entirely, so
            # fwd state needs no zero-init (every read is covered by a prior
            # write). bwd needs zeros for columns entering mid-sequence.
            nc.gpsimd.memset(hS["b"][:, :, :].bitcast(f32 if is_r else mdt), 0.0)
            nc.gpsimd.memset(cS["b"][:, :, :], 0.0)

            def dir_step(d, t, first=False):
                # On a direction's first step h = c = 0: the Whh matmuls
                # vanish and the f gate is unused (c = sig_i * tanh_g).
                c = counts[t]
                gates = (((0, 'i'), (2, 'g'), (3, 'o')) if first else
                         ((0, 'i'), (2, 'g'), (1, 'f'), (3, 'o')))
                for (bs, bw, npad) in col_blocks(c):
                    xs = off[t] + bs
                    sb = {}
                    # PE: gate matmuls; ScalarE evacuates with fused
                    # bias + nonlinearity per 128-row chunk. PSUM tiles are
                    # bank-granular so banks recycle as soon as each chunk
                    # is evacuated.
                    for g, gname in gates:
                        gt = gp.tile([P, 4, 512], mdt, name=f"sb{gname}",
                                     tag=f"g{gname}", bufs=2)
                        func = TANH if g == 2 else SIG
                        for j in range(4):
                            m = g * 4 + j
                            ms = slice(m * P, (m + 1) * P)
                            ps = psp.tile([P, 512], f32, name=f"ps{gname}{j}",
                                          tag="ps", bufs=8)
                            if not first:
                                for k in range(KC):
                                    nc.tensor.matmul(
                                        ps[:, :npad],
                                        whh[d][:, k, ms],
                                        hS[d][:, k, bs:bs + npad],
                                        start=(k == 0), stop=False)
                            nc.tensor.matmul(
                                ps[:, :npad],
                                wih[d][:, ms],
                                xT[:, xs:xs + npad],
                                start=first, stop=True)
                            nc.scalar.activation(
                                gt[:, j, :bw], ps[:, :bw], func,
                                bias=bias[d][:, m:m + 1], scale=1.0)
                        sb[gname] = gt
                    # VectorE cell update, split into two H-chunk halves so
                    # the next step's k-chunk matmuls unblock sooner:
                    #   t1 = sig_i * tanh_g   (in-place into sig_i, fp16 2x)
                    #   c  = sig_f * c + t1   (fp32; first step: c = t1)
                    #   h  = sig_o * tanh(c)  (fp16 out)
                    tc_t = gp.tile([P, 4, 512], mdt, name="tanhc", tag="tc",
                                   bufs=2)
                    for jh in (slice(0, 2), slice(2, 4)):
                        csl = cS[d][:, jh, bs:bs + bw]
                        hsl = hS[d][:, jh, bs:bs + bw]
                        if first:
                            nc.vector.tensor_mul(csl, sb['i'][:, jh, :bw],
                                                 sb['g'][:, jh, :bw])
                        else:
                            nc.vector.tensor_mul(sb['i'][:, jh, :bw],
                                                 sb['i'][:, jh, :bw],
                                                 sb['g'][:, jh, :bw])
                            nc.vector.tensor_mul(csl, sb['f'][:, jh, :bw], csl)
                            nc.vector.tensor_add(csl, csl, sb['i'][:, jh, :bw])
                        nc.scalar.activation(tc_t[:, jh, :bw], csl, TANH)
                        nc.vector.tensor_mul(hsl, sb['o'][:, jh, :bw],
                                             tc_t[:, jh, :bw])

            with tc.tile_pool(name="gates", bufs=2) as gp, \
                 tc.tile_pool(name="psum", bufs=2, space="PSUM") as psp:
                for r in range(L):
                    dir_step('f', r, first=(r == 0))
                    dir_step('b', L - 1 - r, first=(r == 0))

            # Projection: out[words, H] = [h_bwd; h_fwd]^T @ Wp^T + bp
            with tc.tile_pool(name="proj", bufs=2) as prp, \
                 tc.tile_pool(name="prps", bufs=2, space="PSUM") as prps:
                nmc = (m_pad + P - 1) // P
                for mi in range(nmc):
                    pw = min(P, m_pad - mi * P)
                    po = prps.tile([P, H], f32, name="po", tag="po")
                    for kk in range(8):
                        src = hS['b'] if kk < 4 else hS['f']
                        nc.tensor.matmul(
                            po[:pw, :],
                            src[:, kk % 4, mi * P:mi * P + pw],
                            wp[:, kk, :],
                            start=(kk == 0), stop=(kk == 7))
                    ot = prp.tile([P, H], f32, name="ot", tag="ot")
                    nc.vector.tensor_add(ot[:pw, :], po[:pw, :], bp[:pw, :])
                    nc.sync.dma_start(out=out_d[mi * P:mi * P + pw, :],
                                      in_=ot[:pw, :])

    nc.compile()
    return nc


# ---------------------------------------------------------------------------
# Entry points
# ---------------------------------------------------------------------------

def _run(inputs, trace=False, tmpdir=None, reps=1):
    from concourse.bass_utils import run_bass_kernel_spmd

    mmnp = {'float16': np.float16, 'bfloat16': None,
            'float32r': np.float32}.get(MM_DT)
    if mmnp is None:
        import ml_dtypes
        mmnp = ml_dtypes.bfloat16

    chars = np.asarray(inputs["chars"]).reshape(N, L)
    lens = np.asarray(inputs["word_lens"]).reshape(N).astype(np.int64)
    emb = np.asarray(inputs["emb"], dtype=np.float32)

    per_core, counts = _plan(lens)
    m_pad = len(per_core[0])
    ctot = int(sum(counts))
    xcols = ctot + 512

    whh_f, wih_f, bias_f = _prep_weights(
        np.asarray(inputs["Wih_f"], np.float32), np.asarray(inputs["Whh_f"], np.float32),
        np.asarray(inputs["bih_f"], np.float32), np.asarray(inputs["bhh_f"], np.float32),
        mmnp)
    whh_b, wih_b, bias_b = _prep_weights(
        np.asarray(inputs["Wih_b"], np.float32), np.asarray(inputs["Whh_b"], np.float32),
        np.asarray(inputs["bih_b"], np.float32), np.asarray(inputs["bhh_b"], np.float32),
        mmnp)
    Wp = np.asarray(inputs["Wp"], np.float32)
    bp = np.asarray(inputs["bp"], np.float32)
    wp_sb = np.ascontiguousarray(Wp.T.reshape(8, P, H).transpose(1, 0, 2)).astype(mmnp)
    bp_sb = np.ascontiguousarray(np.tile(bp[None, :], (P, 1))).astype(np.float32)

    embT = np.ascontiguousarray(emb.T)  # [128, 256]

    in_maps = []
    for k in range(N_CORES):
        wl = per_core[k]
        cw = np.zeros((m_pad, L), dtype=np.int64)
        for r, wid in enumerate(wl):
            if wid >= 0:
                cw[r] = chars[wid]
        idx = np.concatenate([cw[:counts[t], t] for t in range(L)])
        xT = np.zeros((P, xcols), dtype=mmnp)
        xT[:, :ctot] = embT[:, idx].astype(mmnp)
        in_maps.append(dict(
            xT=xT, whh_f=whh_f, whh_b=whh_b, wih_f=wih_f, wih_b=wih_b,
            bias_f=bias_f, bias_b=bias_b, wp=wp_sb, bp=bp_sb))

    nc = _build_program(counts, m_pad, xcols)
    times = []
    for r in range(reps):
        last = r == reps - 1
        td = (tmpdir + f"_{r}") if (tmpdir and trace) else tmpdir
        res = run_bass_kernel_spmd(nc, in_maps, list(range(N_CORES)),
                                   trace=trace, tmpdir=td)
        times.append(res.exec_time_ns)
    res.all_exec_times = times

    out = np.zeros((N, H), dtype=np.float32)
    for k in range(N_CORES):
        ok = res.results[k]["out"]
        for r, wid in enumerate(per_core[k]):
            if wid >= 0:
                out[wid] = ok[r]
    return out.reshape(B, W, H), res


def kernel(**inputs):
    out, _ = _run(inputs, trace=False)
    return out
